# revision 1
# baseline (speedup 1.0000x reference)
"""Bass/Trainium2 kernel for nn_BlastocystAuxLoss.

Computes a masked MSE over B=16,777,216 elements:
    late stages are labels 8..15; target[s] = (s-8) * 4/7 for late stages;
    loss = sum_{s>=8} (x - target)^2 / count(s>=8)   (0.0 if count == 0)

Strategy: trivially data-parallel over 8 NeuronCores. Each core reads its
B/8 shard of blast_scores (f32) and stage_labels (i32) from HBM, computes
per-partition partial {count, sse} on-chip (DVE + ACT engines, bf16
elementwise math, f32 accumulation), and writes a [128, 2] partial-sums
tile. The final scalar reduction (8*128 partials -> sse/cnt) happens on
host in f64. No collectives needed.

Per-element identities used (s = label, x = score):
    mask  m = (s >= 8)
    target t = relu(s * 4/7 - 32/7)        (== (s-8)*4/7 clamped at 0)
    sse  += (m * (bf16(x) - t))^2          (m^2 == m)
    cnt  += m
"""

from contextlib import ExitStack

import numpy as np

B = 16777216
N_CORES = 8
SHARD = B // N_CORES  # 2,097,152
P = 128

_NC_CACHE = {}


def build(shard=SHARD, n_tiles=8):
    """Build the single-core Bass program (same SPMD program for all cores)."""
    import concourse.bacc as bacc
    import concourse.tile as tile
    from concourse import mybir

    free = shard // P
    fd = free // n_tiles
    assert fd * n_tiles * P == shard

    nc = bacc.Bacc("TRN2", target_bir_lowering=False)
    x_ext = nc.declare_dram_parameter(
        "blast_scores", [shard], mybir.dt.float32, isOutput=False
    )
    s_ext = nc.declare_dram_parameter(
        "stage_labels", [shard], mybir.dt.int32, isOutput=False
    )
    out_ext = nc.declare_dram_parameter("out", [P, 2], mybir.dt.float32, isOutput=True)

    x_v = x_ext.ap().rearrange("(p f) -> p f", p=P)
    s_v = s_ext.ap().rearrange("(p f) -> p f", p=P)

    c47 = 4.0 / 7.0  # target step; folded into the Square's input scale
    c74 = 7.0 / 4.0  # x prescale so z = 7/4*(x - t) uses integer-exact v

    f32 = mybir.dt.float32
    bf16 = mybir.dt.bfloat16
    Alu = mybir.AluOpType
    Act = mybir.ActivationFunctionType

    with tile.TileContext(nc) as tc:
        with (
            tc.tile_pool(name="io", bufs=4) as io_pool,
            tc.tile_pool(name="mid", bufs=3) as mid_pool,
            tc.tile_pool(name="acc", bufs=1) as acc_pool,
        ):
            cnt_acc = acc_pool.tile([P, n_tiles], f32)
            sse_acc = acc_pool.tile([P, n_tiles], f32)
            red = acc_pool.tile([P, 2], f32)
            # bias for the sigmoid step mask: m = sigmoid(64*s - 480)
            sig_bias = acc_pool.tile([P, 1], f32)
            nc.gpsimd.memset(sig_bias[:], -480.0)

            for k in range(n_tiles):
                x_t = io_pool.tile([P, fd], f32, tag="x")
                s_t = io_pool.tile([P, fd], mybir.dt.int32, tag="s")
                nc.sync.dma_start(out=x_t[:], in_=x_v[:, k * fd : (k + 1) * fd])
                nc.sync.dma_start(out=s_t[:], in_=s_v[:, k * fd : (k + 1) * fd])

                m = mid_pool.tile([P, fd], bf16, tag="m")
                v = mid_pool.tile([P, fd], bf16, tag="v")
                z = mid_pool.tile([P, fd], bf16, tag="z")
                zm = mid_pool.tile([P, fd], bf16, tag="zm")
                sq = mid_pool.tile([P, fd], bf16, tag="sq")

                # ACT: step mask m = sigmoid(64*(s - 7.5)) in {0,1} exactly
                # (saturated at +-32); accumulate count for free
                nc.scalar.activation(
                    m[:], s_t[:], Act.Sigmoid, bias=sig_bias[:], scale=64.0,
                    accum_out=cnt_acc[:, k : k + 1],
                )
                # DVE: v = max(s-8, 0)
                nc.vector.tensor_scalar(v[:], s_t[:], 8, 0, Alu.subtract, Alu.max)
                # DVE: z = 7/4*x - v  (== 7/4*(x - target) since v = 7/4*t)
                nc.vector.scalar_tensor_tensor(
                    z[:], x_t[:], c74, v[:], Alu.mult, Alu.subtract
                )
                nc.vector.tensor_tensor(zm[:], z[:], m[:], Alu.mult)
                # ACT: sse += (4/7 * zm)^2 over masked elements
                nc.scalar.activation(
                    sq[:], zm[:], Act.Square, scale=c47,
                    accum_out=sse_acc[:, k : k + 1],
                )

            nc.vector.reduce_sum(red[:, 0:1], cnt_acc[:], axis=mybir.AxisListType.X)
            nc.vector.reduce_sum(red[:, 1:2], sse_acc[:], axis=mybir.AxisListType.X)
            nc.sync.dma_start(out=out_ext.ap()[:, :], in_=red[:])

    nc.finalize()
    return nc


def build_raw(shard=2097152, sizes=None, ring=6):
    """Hand-scheduled raw-Bass builder (no TileContext).

    - per-slot DMA semaphores (multi-queue completions are unordered);
      slot reuse (tile k vs k+R) is ordered by issue-side consumer waits
    - ring of 6 slots so DMA issue never gates on compute and the input
      stream stays bandwidth-bound end to end
    - tile sizes taper at the end so the last tile's compute lag after
      the final (bandwidth-bound) DMA is minimal
    - final reduction via a TensorEngine ones-matmul (cross-partition sum
      -> PSUM [1, 2*NT]) so the output DMA is one small descriptor instead
      of 128 8-byte ones
    """
    import concourse.bacc as bacc
    from concourse import mybir

    free = shard // P
    if sizes is None:
        sizes = [2048] * 7 + [1536, 512]
        if sum(sizes) != free:  # non-default shard (tests)
            fd = free // 8
            sizes = [fd] * 8
    assert sum(sizes) == free
    fd = max(sizes)
    NT = len(sizes)
    offs = [sum(sizes[:i]) for i in range(NT)]
    R = min(ring, NT)

    nc = bacc.Bacc("TRN2", target_bir_lowering=False)
    x_ext = nc.declare_dram_parameter(
        "blast_scores", [shard], mybir.dt.float32, isOutput=False
    )
    s_ext = nc.declare_dram_parameter(
        "stage_labels", [shard], mybir.dt.int32, isOutput=False
    )
    out_ext = nc.declare_dram_parameter("out", [2 * NT], mybir.dt.float32, isOutput=True)

    x_v = x_ext.ap().rearrange("(p f) -> p f", p=P)
    s_v = s_ext.ap().rearrange("(p f) -> p f", p=P)

    c47 = 4.0 / 7.0
    c74 = 7.0 / 4.0

    f32 = mybir.dt.float32
    i32 = mybir.dt.int32
    bf16 = mybir.dt.bfloat16
    Alu = mybir.AluOpType
    Act = mybir.ActivationFunctionType

    x_t = [nc.alloc_sbuf_tensor(f"x{i}", [P, fd], f32).ap() for i in range(R)]
    s_t = [nc.alloc_sbuf_tensor(f"s{i}", [P, fd], i32).ap() for i in range(R)]
    m_t = [nc.alloc_sbuf_tensor(f"m{i}", [P, fd], bf16).ap() for i in range(R)]
    v_t = [nc.alloc_sbuf_tensor(f"v{i}", [P, fd], bf16).ap() for i in range(2)]
    z_t = [nc.alloc_sbuf_tensor(f"z{i}", [P, fd], bf16).ap() for i in range(2)]
    zm_t = [nc.alloc_sbuf_tensor(f"zm{i}", [P, fd], bf16).ap() for i in range(R)]
    sq_t = nc.alloc_sbuf_tensor("sq", [P, fd], bf16).ap()
    # acc[:, k] = per-partition count of tile k; acc[:, NT+k] = partial sse
    acc = nc.alloc_sbuf_tensor("acc", [P, 2 * NT], f32).ap()
    red1 = nc.alloc_sbuf_tensor("red1", [1, 2 * NT], f32).ap()
    sig_bias = nc.alloc_sbuf_tensor("sig_bias", [P, 1], f32).ap()
    ones = nc.const_aps.tensor(1.0, (P, 1), f32)

    with ExitStack() as ctx:
        dma_x = [ctx.enter_context(nc.semaphore(f"dma_x{i}")) for i in range(R)]
        dma_s = [ctx.enter_context(nc.semaphore(f"dma_s{i}")) for i in range(R)]
        dve = ctx.enter_context(nc.semaphore("dve"))
        act = ctx.enter_context(nc.semaphore("act"))
        mm = ctx.enter_context(nc.semaphore("mm"))
        outd = ctx.enter_context(nc.semaphore("outd"))
        bias_rdy = ctx.enter_context(nc.semaphore("bias_rdy"))
        psum = ctx.enter_context(nc.psum_tensor("ps", [1, 2 * NT], f32))
        block = ctx.enter_context(nc.Block())

        # Semaphore increment ledger:
        #   DVE: 3 per tile (v, z, zm)            -> 3*NT total
        #   ACT: 2 per tile (m, sq) + final copy  -> 2*NT + 1 total
        #   DMA slot sems: +16 per transfer into that slot

        @block.sync
        def _(sync):
            for k in range(NT):
                i = k % R
                w = sizes[k]
                if k >= R:
                    # x slot free when z(k-R) done; s slot free when
                    # v(k-R) (implied by z) and m(k-R) done
                    sync.wait_ge(dve, 3 * (k - R) + 2)
                    sync.wait_ge(act, 2 * (k - R) + 1)
                sync.dma_start(
                    out=s_t[i][:, :w], in_=s_v[:, offs[k] : offs[k] + w]
                ).then_inc(dma_s[i], 16)
                sync.dma_start(
                    out=x_t[i][:, :w], in_=x_v[:, offs[k] : offs[k] + w]
                ).then_inc(dma_x[i], 16)
            sync.wait_ge(act, 2 * NT + 1)  # final ScE copy done
            sync.dma_start(out=out_ext.ap()[:], in_=red1[0:1, :]).then_inc(outd, 16)
            sync.wait_ge(outd, 16)

        @block.vector
        def _(vector):
            vector.memset(sig_bias[:, :], -480.0).then_inc(bias_rdy, 1)
            for k in range(NT):
                i = k % R
                w = sizes[k]
                rnd = 16 * (k // R + 1)
                # v = max(s-8, 0)
                vector.wait_ge(dma_s[i], rnd)
                vector.tensor_scalar(
                    v_t[k % 2][:, :w], s_t[i][:, :w], 8, 0, Alu.subtract, Alu.max
                ).then_inc(dve, 1)
                # z = 7/4*x - v
                vector.wait_ge(dma_x[i], rnd)
                vector.wait_ge(dve, 3 * k + 1)  # v(k) drained
                vector.scalar_tensor_tensor(
                    z_t[k % 2][:, :w], x_t[i][:, :w], c74, v_t[k % 2][:, :w],
                    Alu.mult, Alu.subtract,
                ).then_inc(dve, 1)
                # zm = z * m   (m(k) ready when act >= 2k+1)
                vector.wait_ge(act, 2 * k + 1)
                vector.wait_ge(dve, 3 * k + 2)  # z(k) drained
                vector.tensor_tensor(
                    zm_t[i][:, :w], z_t[k % 2][:, :w], m_t[i][:, :w], Alu.mult
                ).then_inc(dve, 1)

        @block.scalar
        def _(scalar):
            scalar.wait_ge(bias_rdy, 1)
            for k in range(NT):
                i = k % R
                w = sizes[k]
                rnd = 16 * (k // R + 1)
                # m = sigmoid(64*s - 480) in {0,1}; count accumulates free
                scalar.wait_ge(dma_s[i], rnd)
                if k >= R:
                    # m slot free when zm(k-R) done
                    scalar.wait_ge(dve, 3 * (k - R) + 3)
                scalar.activation(
                    m_t[i][:, :w], s_t[i][:, :w], Act.Sigmoid,
                    bias=sig_bias[:, :], scale=64.0,
                    accum_out=acc[:, k : k + 1],
                ).then_inc(act, 1)
                # sq = Square(zm * 4/7); sse accum; zm(k): dve >= 3k+3
                scalar.wait_ge(dve, 3 * k + 3)
                scalar.activation(
                    sq_t[:, :w], zm_t[i][:, :w], Act.Square, scale=c47,
                    accum_out=acc[:, NT + k : NT + k + 1],
                ).then_inc(act, 1)
            # after the matmul: PSUM -> SBUF single-partition copy, then
            # ship the 2*NT partials out (single 8*2*NT-byte descriptor);
            # issuing here avoids a cross-engine hop before the final DMA
            scalar.wait_ge(mm, 1)
            scalar.activation(red1[0:1, :], psum.ap()[0:1, :], Act.Copy).then_inc(
                act, 1
            )

        @block.tensor
        def _(tensor):
            # cross-partition reduction: ones.T @ acc -> [1, 2*NT]
            tensor.wait_ge(act, 2 * NT)
            tensor.wait_ge(dve, 3 * NT)
            tensor.matmul(psum.ap()[0:1, :], ones, acc[:, :]).then_inc(mm, 1)

    nc.finalize()
    return nc


def run(x, s, **spmd_kwargs):
    """Shard, run on 8 cores, host-reduce. Returns (loss, BassKernelResults)."""
    from concourse.bass_utils import run_bass_kernel_spmd

    if "nc" not in _NC_CACHE:
        _NC_CACHE["nc"] = build_raw()
    nc = _NC_CACHE["nc"]

    in_maps = [
        {
            "blast_scores": x[i * SHARD : (i + 1) * SHARD],
            "stage_labels": s[i * SHARD : (i + 1) * SHARD],
        }
        for i in range(N_CORES)
    ]
    res = run_bass_kernel_spmd(nc, in_maps, core_ids=list(range(N_CORES)), **spmd_kwargs)

    cnt = 0.0
    sse = 0.0
    for r in res.results:
        o = r["out"].astype(np.float64).reshape(2, -1)
        cnt += o[0].sum()
        sse += o[1].sum()
    val = sse / max(cnt, 1.0) if cnt > 0 else 0.0
    return np.asarray(val, dtype=np.float32), res


def kernel(**inputs):
    x = np.ascontiguousarray(np.asarray(inputs["blast_scores"], dtype=np.float32))
    s = np.ascontiguousarray(np.asarray(inputs["stage_labels"], dtype=np.int32))
    assert x.shape == (B,) and s.shape == (B,)
    return run(x, s)[0]



# revision 2
# speedup vs baseline: 1.0805x; 1.0805x over previous
"""Optimized Bass/Trainium2 kernel for nn_BlastocystAuxLoss (v2).

Same op set as the baseline, but software-pipelined so compute keeps
pace with the DMA stream (~4.88us per 2048-col tile pair at ~420 GB/s):

  per tile k (width w):
    DVE : v  = max(s-8, 0)         [ts, i32 -> bf16]
          z  = 7/4*x - v           [stt, f32 in -> bf16]
          zm = z * m               [tt bf16]
    ACT : m  = sigmoid(64*s-480)   in {0,1} exactly; accum -> cnt
          sq = Square(4/7 * zm)    accum -> sse

  The baseline issued ACT ops in-tile order (m(k), sq(k)), which chains
  m(k) -> zm(k) -> sq(k) across engines serially: ~5.5us/tile cadence,
  slower than the 4.88us DMA delivery, so compute lagged ~14us past the
  stream. Here ACT runs m one tile ahead of sq:
      m0, m1, sq0, m2, sq1, ..., m(NT-1), sq(NT-2), sq(NT-1)
  which breaks the chain; per-tile busy is DVE ~4.74us / ACT ~4.53us,
  both under the DMA cadence -> DMA-streaming-bound end to end.

  Also: activation tables are pre-warmed before the first s-tile lands
  (the table load cost ~1.3us on the critical path in the baseline),
  the first tile is smaller so compute starts earlier, and the tail
  tapers so the post-stream drain is minimal.

Final reduction: ones^T @ acc matmul -> PSUM [1, 2*NT] -> single small
output DMA; host reduces 8 cores in f64.
"""

from contextlib import ExitStack

import numpy as np

B = 16777216
N_CORES = 8
SHARD = B // N_CORES  # 2,097,152
P = 128

_NC_CACHE = {}


def build_raw2(shard=2097152, sizes=None, ring=6):
    import concourse.bacc as bacc
    from concourse import mybir

    free = shard // P
    if sizes is None:
        sizes = [2048] * 7 + [1536, 512]
        if sum(sizes) != free:  # non-default shard (sim tests)
            fd = free // 8
            sizes = [fd] * 8
    assert sum(sizes) == free
    fd = max(sizes)
    NT = len(sizes)
    offs = [sum(sizes[:i]) for i in range(NT)]
    R = min(ring, NT)

    nc = bacc.Bacc("TRN2", target_bir_lowering=False)
    x_ext = nc.declare_dram_parameter(
        "blast_scores", [shard], mybir.dt.float32, isOutput=False
    )
    s_ext = nc.declare_dram_parameter(
        "stage_labels", [shard], mybir.dt.int32, isOutput=False
    )
    out_ext = nc.declare_dram_parameter(
        "out", [P, 2 * NT], mybir.dt.float32, isOutput=True
    )

    x_v = x_ext.ap().rearrange("(p f) -> p f", p=P)
    s_v = s_ext.ap().rearrange("(p f) -> p f", p=P)

    c47 = 4.0 / 7.0
    c74 = 7.0 / 4.0

    f32 = mybir.dt.float32
    i32 = mybir.dt.int32
    bf16 = mybir.dt.bfloat16
    Alu = mybir.AluOpType
    Act = mybir.ActivationFunctionType

    x_t = [nc.alloc_sbuf_tensor(f"x{i}", [P, fd], f32).ap() for i in range(R)]
    s_t = [nc.alloc_sbuf_tensor(f"s{i}", [P, fd], i32).ap() for i in range(R)]
    m_t = [nc.alloc_sbuf_tensor(f"m{i}", [P, fd], bf16).ap() for i in range(R)]
    v_t = [nc.alloc_sbuf_tensor(f"v{i}", [P, fd], bf16).ap() for i in range(2)]
    z_t = [nc.alloc_sbuf_tensor(f"z{i}", [P, fd], bf16).ap() for i in range(2)]
    zm_t = [nc.alloc_sbuf_tensor(f"zm{i}", [P, fd], bf16).ap() for i in range(R)]
    sq_t = [nc.alloc_sbuf_tensor(f"sq{i}", [P, fd], bf16).ap() for i in range(2)]
    # acc[:, k] = per-partition count of tile k; acc[:, NT+k] = partial sse
    acc = nc.alloc_sbuf_tensor("acc", [P, 2 * NT], f32).ap()
    warm = nc.alloc_sbuf_tensor("warm", [P, 1], f32).ap()
    warm1 = nc.alloc_sbuf_tensor("warm1", [P, 1], f32).ap()
    warm2 = nc.alloc_sbuf_tensor("warm2", [P, 1], f32).ap()
    sig_bias = nc.alloc_sbuf_tensor("sig_bias", [P, 1], f32).ap()

    # act-sem increment position of m(k) / sq(k) in the pipelined order:
    #   m0, m1, sq0, m2, sq1, ..., m(NT-1), sq(NT-2), sq(NT-1)
    def m_pos(k):
        return 1 if k == 0 else 2 * k

    def sq_pos(k):
        return 2 * NT if k == NT - 1 else 2 * k + 3

    with ExitStack() as ctx:
        dma_x = [ctx.enter_context(nc.semaphore(f"dma_x{i}")) for i in range(R)]
        dma_s = [ctx.enter_context(nc.semaphore(f"dma_s{i}")) for i in range(R)]
        dve = ctx.enter_context(nc.semaphore("dve"))
        act = ctx.enter_context(nc.semaphore("act"))
        outd = ctx.enter_context(nc.semaphore("outd"))
        warm_rdy = ctx.enter_context(nc.semaphore("warm_rdy"))
        block = ctx.enter_context(nc.Block())

        # Semaphore increment ledger:
        #   DVE: 3 per tile (v, z, zm)                  -> 3*NT total
        #   ACT: 2 per tile (m, sq; pipelined order)    -> 2*NT total
        #   DMA slot sems: +16 per transfer into that slot

        @block.sync
        def _(sync):
            for k in range(NT):
                i = k % R
                w = sizes[k]
                if k >= R:
                    # s slot free when v(k-R) (DVE, implied by z) and m(k-R)
                    # (ACT) done; x slot free when z(k-R) done
                    sync.wait_ge(dve, 3 * (k - R) + 2)
                    sync.wait_ge(act, m_pos(k - R))
                sync.dma_start(
                    out=s_t[i][:, :w], in_=s_v[:, offs[k] : offs[k] + w]
                ).then_inc(dma_s[i], 16)
                sync.dma_start(
                    out=x_t[i][:, :w], in_=x_v[:, offs[k] : offs[k] + w]
                ).then_inc(dma_x[i], 16)
            # ship the raw [P, 2*NT] partials; host does the final 128-way
            # sum. act >= 2*NT (last sq accum read) covers every acc write
            # (each acc column lands via an ACT accumulator read).
            sync.wait_ge(act, 2 * NT)
            sync.dma_start(out=out_ext.ap()[:, :], in_=acc[:, :]).then_inc(outd, 16)
            sync.wait_ge(outd, 16)

        @block.vector
        def _(vector):
            vector.memset(warm[:, :], 0.0).then_inc(warm_rdy, 1)
            vector.memset(sig_bias[:, :], -480.0).then_inc(warm_rdy, 1)
            for k in range(NT):
                i = k % R
                w = sizes[k]
                rnd = 16 * (k // R + 1)
                # v = max(s-8, 0)
                vector.wait_ge(dma_s[i], rnd)
                if k >= 2:
                    # v buffer (2-deep) free when z(k-2) drained
                    vector.wait_ge(dve, 3 * (k - 2) + 2)
                vector.tensor_scalar(
                    v_t[k % 2][:, :w], s_t[i][:, :w], 8, 0, Alu.subtract, Alu.max
                ).then_inc(dve, 1)
                # z = 7/4*x - v
                vector.wait_ge(dma_x[i], rnd)
                vector.wait_ge(dve, 3 * k + 1)  # v(k) drained
                vector.scalar_tensor_tensor(
                    z_t[k % 2][:, :w], x_t[i][:, :w], c74, v_t[k % 2][:, :w],
                    Alu.mult, Alu.subtract,
                ).then_inc(dve, 1)
                # zm = z * m   (m(k) from ACT, issued one tile ahead)
                vector.wait_ge(dve, 3 * k + 2)  # z(k) drained
                vector.wait_ge(act, m_pos(k))
                if k >= R:
                    # zm slot free when sq(k-R) done
                    vector.wait_ge(act, sq_pos(k - R))
                vector.tensor_tensor(
                    zm_t[i][:, :w], z_t[k % 2][:, :w], m_t[i][:, :w], Alu.mult
                ).then_inc(dve, 1)

        @block.scalar
        def _(scalar):
            # pre-warm the activation tables before the first s-tile lands;
            # in the baseline the Sigmoid table load (~1.3us) sat on the
            # critical path between the first DMA and the first ACT op
            scalar.wait_ge(warm_rdy, 2)
            scalar.activation(warm1[:, :], warm[:, :], Act.Sigmoid, bias=sig_bias[:, :])
            scalar.activation(warm2[:, :], warm[:, :], Act.Square)

            def m_op(k):
                i = k % R
                w = sizes[k]
                rnd = 16 * (k // R + 1)
                scalar.wait_ge(dma_s[i], rnd)
                if k >= R:
                    # m slot free when zm(k-R) done
                    scalar.wait_ge(dve, 3 * (k - R) + 3)
                scalar.activation(
                    m_t[i][:, :w], s_t[i][:, :w], Act.Sigmoid,
                    bias=sig_bias[:, :], scale=64.0,
                    accum_out=acc[:, k : k + 1],
                ).then_inc(act, 1)

            def sq_op(k):
                i = k % R
                w = sizes[k]
                scalar.wait_ge(dve, 3 * k + 3)  # zm(k) ready
                if k >= 2:
                    # sq buffer (2-deep) WAW ordering contract
                    scalar.wait_ge(act, sq_pos(k - 2))
                scalar.activation(
                    sq_t[k % 2][:, :w], zm_t[i][:, :w], Act.Square, scale=c47,
                    accum_out=acc[:, NT + k : NT + k + 1],
                ).then_inc(act, 1)

            # software-pipelined order: m runs one tile ahead of sq
            m_op(0)
            for k in range(1, NT):
                m_op(k)
                sq_op(k - 1)
            sq_op(NT - 1)

    nc.finalize()
    return nc


def run(x, s, **spmd_kwargs):
    """Shard, run on 8 cores, host-reduce. Returns (loss, BassKernelResults)."""
    from concourse.bass_utils import run_bass_kernel_spmd

    if "nc" not in _NC_CACHE:
        _NC_CACHE["nc"] = build_raw2()
    nc = _NC_CACHE["nc"]

    in_maps = [
        {
            "blast_scores": x[i * SHARD : (i + 1) * SHARD],
            "stage_labels": s[i * SHARD : (i + 1) * SHARD],
        }
        for i in range(N_CORES)
    ]
    res = run_bass_kernel_spmd(nc, in_maps, core_ids=list(range(N_CORES)), **spmd_kwargs)

    cnt = 0.0
    sse = 0.0
    for r in res.results:
        o = r["out"].astype(np.float64)  # [P, 2*NT]
        nt = o.shape[1] // 2
        cnt += o[:, :nt].sum()
        sse += o[:, nt:].sum()
    val = sse / max(cnt, 1.0) if cnt > 0 else 0.0
    return np.asarray(val, dtype=np.float32), res


def kernel(**inputs):
    x = np.ascontiguousarray(np.asarray(inputs["blast_scores"], dtype=np.float32))
    s = np.ascontiguousarray(np.asarray(inputs["stage_labels"], dtype=np.int32))
    assert x.shape == (B,) and s.shape == (B,)
    return run(x, s)[0]
